# revision 8
# baseline (speedup 1.0000x reference)
"""ConvTranspose3d(64->32, k=3, stride=2, pad=1, out_pad=1, dilation=2) on 8 NeuronCores.

Math: with stride=2, dilation=2, padding=1, k=3, output position o = 2i + 2k - 1
is odd in every spatial dim, so the transposed conv collapses to a dense 3^3
conv y = conv3d(x, wc, padding=1) on the 32^3 grid (wc = flip(transpose(w))),
scattered into the odd sub-lattice of the 66^3 output; every other output
voxel is just bias. Verified exact vs the reference.

Sharding: 8 shards = 2 batches x 4 depth-blocks of 8 conv planes. Each core
computes its depth slab with an implicit GEMM: M = (c_out=32 x 4 depth planes)
on PSUM partitions, K = (64 c_in x 2 input planes) on SBUF partitions via a
block-Toeplitz-over-depth stationary operand (host-built), N = 512 hw pixels
per matmul, accumulating 27 matmuls (9 hw taps x 3 K-chunks) + one K=1
bias matmul per PSUM bank in float32r.

Perf notes (from perfetto): each dma_start costs ~650ns of sequencer issue
time, so issues are spread across all five engine queues; warmup matmuls ramp the PE p-state during the input-DMA
window. The device ships only the 32^3 conv voxels (1.05MB/core); all other
output voxels are bias, filled host-side during unshard.
"""

import sys

sys.path.insert(0, "/opt/trn_rl_repo")

import numpy as np

N_CORES = 8
D_BLOCKS = 4  # depth blocks per batch
G_PER_CORE = 8  # conv output planes per core

_cache = {}


def _build_nc():
    import concourse.bass as bass
    import concourse.tile as tile
    from concourse import bacc, mybir

    dt = mybir.dt
    nc = bacc.Bacc("TRN2", target_bir_lowering=False, debug=False,
                   num_devices=N_CORES)

    # xs: 5 pairs of adjacent (zero-padded) input depth planes; partition
    # p = dpi*64 + ci. tcw: 27 block-Toeplitz stationary matrices, columns
    # (chunk*9 + tap)*128 + (co*4 + gb). bias: p = co*4+gb -> bias[co].
    xs = nc.dram_tensor("xs", [5, 128, 34, 34], dt.float32r,
                        kind="ExternalInput")
    tcw = nc.dram_tensor("tcw", [128, 27 * 128], dt.float32r,
                         kind="ExternalInput")
    bias = nc.dram_tensor("bias", [128, 1], dt.float32,
                          kind="ExternalInput")
    # conv voxels only: partition (co*4+gb), group g = b*2+hh, px = (16h, 32w)
    out = nc.dram_tensor("out", [128, 4, 512], dt.float32,
                         kind="ExternalOutput")

    with tile.TileContext(nc) as tc:
        with (
            tc.tile_pool(name="tw", bufs=1) as tw_pool,
            tc.tile_pool(name="xp", bufs=1) as xp_pool,
            tc.tile_pool(name="bc", bufs=1) as bc_pool,
            tc.tile_pool(name="ot", bufs=4) as ot_pool,
            tc.tile_pool(name="dm", bufs=1) as dm_pool,
            tc.tile_pool(name="ps", bufs=4, space="PSUM") as ps_pool,
            tc.tile_pool(name="wps", bufs=1, space="PSUM") as wps_pool,
        ):
            tw_t = tw_pool.tile([128, 27 * 128], dt.float32r)
            xp = []
            for p in range(5):
                xp_tile = xp_pool.tile([128, 34, 34], dt.float32r,
                                       tag=f"xp{p}")
                xp.append(xp_tile)
            bias_t = bc_pool.tile([128, 1], dt.float32)
            dummy = dm_pool.tile([128, 512], dt.bfloat16)

            def load_tw(lo, hi, eng):
                eng.dma_start(tw_t[:, lo * 128:hi * 128],
                              tcw[:, lo * 128:hi * 128])

            def load_xp(p, rows, eng):
                eng.dma_start(xp[p][:, rows, :], xs[p, :, rows, :])

            # ~650ns of sequencer time per dma_start issue: spread the input
            # loads across engine queues, in first-use order per queue.
            # Sync: Toeplitz weights (first matmul needs only block 0).
            load_tw(0, 1, nc.sync)
            load_tw(1, 9, nc.sync)
            load_tw(9, 18, nc.sync)
            load_tw(18, 27, nc.sync)
            # Scalar: x pairs 0-2 (used by groups b=0 and the start of b=1).
            load_xp(0, slice(0, 18), nc.scalar)
            load_xp(0, slice(18, 34), nc.scalar)
            load_xp(1, slice(0, 34), nc.scalar)
            load_xp(2, slice(0, 34), nc.scalar)
            # Vector only does the warmup-dummy memset (it cannot issue DMAs).
            nc.vector.memset(dummy[:], 0.0)
            # GpSimd: constants + late x pairs, later the output stores.
            nc.gpsimd.dma_start(bias_t[:], bias[:])
            load_xp(3, slice(0, 34), nc.gpsimd)
            load_xp(4, slice(0, 34), nc.gpsimd)

            # warmup matmuls on zeroed garbage ramp the PE p-state during
            # the input-DMA window so the real matmuls run at full clock
            wps = wps_pool.tile([128, 512], dt.float32)
            warm_mm = None
            for _ in range(4):
                warm_mm = nc.tensor.matmul(wps[:], dummy[:, 0:128], dummy[:],
                                           start=True, stop=True)

            prev_last_mm = warm_mm
            for b in range(2):
                for hh in range(2):
                    g = b * 2 + hh
                    h0 = 16 * hh
                    ps = ps_pool.tile([128, 16, 32], dt.float32)
                    i = 0
                    for c in range(3):
                        src = xp[2 * b + c]
                        for t9 in range(9):
                            kh, kw = t9 // 3, t9 % 3
                            lhsT = tw_t[:, (c * 9 + t9) * 128:
                                        (c * 9 + t9 + 1) * 128]
                            rhs = src[:, h0 + kh:h0 + kh + 16, kw:kw + 32]
                            mm = nc.tensor.matmul(ps[:], lhsT, rhs,
                                                  start=(i == 0),
                                                  stop=(i == 26))
                            # keep the PE's static order group-contiguous so
                            # each store fires right after its last matmul
                            if i == 0 and prev_last_mm is not None:
                                tile.add_dep_helper(
                                    mm.ins, prev_last_mm.ins, sync=False,
                                    reason="group-contiguous PE order")
                            i += 1
                    prev_last_mm = mm
                    # PSUM cannot be DMA'd directly: fused bias-add + copy
                    # to SBUF on DVE, then store from GpSimd's queue
                    ot_g = ot_pool.tile([128, 512], dt.float32, tag=f"ot{g}")
                    nc.vector.tensor_scalar_add(ot_g[:], ps[:], bias_t[:])
                    nc.gpsimd.dma_start(out[:, g, :], ot_g[:])

    nc.compile()
    return nc


def _prep_shared(weight, bias):
    # wc[co, ci, kd, kh, kw] = weight[ci, co, 2-kd, 2-kh, 2-kw]
    wc = np.flip(np.transpose(weight, (1, 0, 2, 3, 4)), axis=(2, 3, 4))
    # full pre-built Toeplitz: tcw[dpi*64+ci, (c*9+t)*128 + co*4 + gb]
    tcw = np.zeros((128, 27, 128), np.float32)
    for c in range(3):
        for dpi in range(2):
            for gb in range(4):
                kd = 2 * c + dpi - gb
                if 0 <= kd <= 2:
                    arr = wc[:, :, kd].reshape(32, 64, 9).transpose(1, 2, 0)
                    tcw[dpi * 64:(dpi + 1) * 64,
                        c * 9:(c + 1) * 9, gb::4] = arr
    tcw = np.ascontiguousarray(tcw.reshape(128, 27 * 128))
    bias128 = np.ascontiguousarray(
        np.repeat(bias.astype(np.float32), 4).reshape(128, 1))
    return tcw, bias128


def _make_slab(x, n, cblk):
    # 5 pairs of spatially padded planes (34x34, zero border);
    # pair p = unpadded planes (8c-1+2p, 8c+2p)
    xs = np.zeros((5, 128, 34, 34), np.float32)
    lo = G_PER_CORE * cblk - 1
    for p in range(5):
        for dpi in range(2):
            d = lo + 2 * p + dpi
            if 0 <= d < 32:
                xs[p, dpi * 64:(dpi + 1) * 64, 1:33, 1:33] = x[n, :, d]
    return xs


def _make_in_maps(x, weight, bias):
    tcw, bias128 = _prep_shared(weight, bias)
    in_maps = []
    for core in range(N_CORES):
        n, cblk = divmod(core, D_BLOCKS)
        in_maps.append({"xs": _make_slab(x, n, cblk), "tcw": tcw,
                        "bias": bias128})
    return in_maps


def kernel(x, weight, bias):
    from concourse.bass_utils import run_bass_kernel_spmd

    if "nc" not in _cache:
        _cache["nc"] = _build_nc()
    nc = _cache["nc"]

    x = np.asarray(x, np.float32)
    weight = np.asarray(weight, np.float32)
    bias = np.asarray(bias, np.float32)

    in_maps = _make_in_maps(x, weight, bias)
    res = run_bass_kernel_spmd(nc, in_maps, core_ids=list(range(N_CORES)))

    # every non-conv voxel (even lattice positions, trailing output_padding
    # planes) is exactly bias; fill host-side and scatter the conv voxels
    # into the odd sub-lattice
    full = np.empty((2, 32, 66, 66, 66), np.float32)
    full[:] = bias.reshape(1, 32, 1, 1, 1)
    for core in range(N_CORES):
        n, cblk = divmod(core, D_BLOCKS)
        # [128, 4, 512] -> (co, gb, b, hh, h, w) -> (co, q=4b+gb, 16hh+h, w)
        arr = res.results[core]["out"].reshape(32, 4, 2, 2, 16, 32)
        arr = arr.transpose(0, 2, 1, 3, 4, 5).reshape(32, 8, 32, 32)
        d0 = 16 * cblk
        full[n, :, d0 + 1:d0 + 17:2, 1:65:2, 1:65:2] = arr
    return full


# revision 9
# speedup vs baseline: 1.0943x; 1.0943x over previous
"""ConvTranspose3d(64->32, k=3, stride=2, pad=1, out_pad=1, dilation=2) on 8 NeuronCores.

Math: with stride=2, dilation=2, padding=1, k=3, output position o = 2i + 2k - 1
is odd in every spatial dim, so the transposed conv collapses to a dense 3^3
conv y = conv3d(x, wc, padding=1) on the 32^3 grid (wc = flip(transpose(w))),
scattered into the odd sub-lattice of the 66^3 output; every other output
voxel is just bias. Verified exact vs the reference.

Sharding: 8 shards = 2 batches x 4 depth-blocks of 8 conv planes. Each core
computes its depth slab with an implicit GEMM: M = (c_out=32 x 4 depth planes)
on PSUM partitions, K = (64 c_in x 2 input planes) on SBUF partitions via a
block-Toeplitz-over-depth stationary operand (host-built), N = 512 hw pixels
per matmul, accumulating 27 matmuls (9 hw taps x 3 K-chunks) + one K=1
bias matmul per PSUM bank in float32r.

Perf notes (from perfetto): each dma_start costs ~650ns of sequencer issue
time, so issues are spread across all five engine queues; warmup matmuls ramp the PE p-state during the input-DMA
window. The device ships only the 32^3 conv voxels (1.05MB/core); all other
output voxels are bias, filled host-side during unshard.
"""

import sys

sys.path.insert(0, "/opt/trn_rl_repo")

import numpy as np

N_CORES = 8
D_BLOCKS = 4  # depth blocks per batch
G_PER_CORE = 8  # conv output planes per core

_cache = {}


def _build_nc():
    import concourse.bass as bass
    import concourse.tile as tile
    from concourse import bacc, mybir

    dt = mybir.dt
    nc = bacc.Bacc("TRN2", target_bir_lowering=False, debug=False,
                   num_devices=N_CORES)

    # xs: 5 pairs of adjacent (zero-padded) input depth planes; partition
    # p = dpi*64 + ci. tcw: 27 block-Toeplitz stationary matrices, columns
    # (chunk*9 + tap)*128 + (co*4 + gb). bias: p = co*4+gb -> bias[co].
    xs = nc.dram_tensor("xs", [5, 128, 34, 34], dt.float32r,
                        kind="ExternalInput")
    tcw = nc.dram_tensor("tcw", [128, 27 * 128], dt.float32r,
                         kind="ExternalInput")
    bias = nc.dram_tensor("bias", [128, 1], dt.float32,
                          kind="ExternalInput")
    # conv voxels only: partition (co*4+gb), group g = b*2+hh, px = (16h, 32w)
    out = nc.dram_tensor("out", [128, 4, 512], dt.float32,
                         kind="ExternalOutput")

    with tile.TileContext(nc) as tc:
        with (
            tc.tile_pool(name="tw", bufs=1) as tw_pool,
            tc.tile_pool(name="xp", bufs=1) as xp_pool,
            tc.tile_pool(name="bc", bufs=1) as bc_pool,
            tc.tile_pool(name="ot", bufs=4) as ot_pool,
            tc.tile_pool(name="dm", bufs=1) as dm_pool,
            tc.tile_pool(name="ps", bufs=4, space="PSUM") as ps_pool,
            tc.tile_pool(name="wps", bufs=1, space="PSUM") as wps_pool,
        ):
            tw_t = tw_pool.tile([128, 27 * 128], dt.float32r)
            xp = []
            for p in range(5):
                xp_tile = xp_pool.tile([128, 34, 34], dt.float32r,
                                       tag=f"xp{p}")
                xp.append(xp_tile)
            bias_t = bc_pool.tile([128, 1], dt.float32)
            dummy = dm_pool.tile([128, 512], dt.bfloat16)

            def load_tw(lo, hi, eng):
                eng.dma_start(tw_t[:, lo * 128:hi * 128],
                              tcw[:, lo * 128:hi * 128])

            def load_xp(p, rows, eng):
                eng.dma_start(xp[p][:, rows, :], xs[p, :, rows, :])

            # ~650ns of sequencer time per dma_start issue: spread the input
            # loads across the three DMA-capable queues, strictly in
            # first-use order per queue so the PE never starves.
            # Sync: Toeplitz weights (first matmul needs only block 0).
            load_tw(0, 1, nc.sync)
            load_tw(1, 9, nc.sync)
            load_tw(9, 18, nc.sync)
            load_tw(18, 27, nc.sync)
            # Scalar: x pairs 0-2 halves (groups b=0 and the start of b=1).
            load_xp(0, slice(0, 18), nc.scalar)
            load_xp(1, slice(0, 18), nc.scalar)
            load_xp(2, slice(0, 18), nc.scalar)
            load_xp(0, slice(18, 34), nc.scalar)
            load_xp(1, slice(18, 34), nc.scalar)
            load_xp(2, slice(18, 34), nc.scalar)
            # Vector only does the warmup-dummy memset (it cannot issue DMAs).
            nc.vector.memset(dummy[:], 0.0)
            # GpSimd: constants + late x pairs, later the output stores.
            nc.gpsimd.dma_start(bias_t[:], bias[:])
            load_xp(3, slice(0, 18), nc.gpsimd)
            load_xp(4, slice(0, 18), nc.gpsimd)
            load_xp(3, slice(18, 34), nc.gpsimd)
            load_xp(4, slice(18, 34), nc.gpsimd)

            # warmup matmuls on zeroed garbage ramp the PE p-state during
            # the input-DMA window so the real matmuls run at full clock
            wps = wps_pool.tile([128, 512], dt.float32)
            warm_mm = None
            for _ in range(3):
                warm_mm = nc.tensor.matmul(wps[:], dummy[:, 0:128], dummy[:],
                                           start=True, stop=True)

            prev_last_mm = warm_mm
            for b in range(2):
                for hh in range(2):
                    g = b * 2 + hh
                    h0 = 16 * hh
                    ps = ps_pool.tile([128, 16, 32], dt.float32)
                    i = 0
                    for c in range(3):
                        src = xp[2 * b + c]
                        for t9 in range(9):
                            kh, kw = t9 // 3, t9 % 3
                            lhsT = tw_t[:, (c * 9 + t9) * 128:
                                        (c * 9 + t9 + 1) * 128]
                            rhs = src[:, h0 + kh:h0 + kh + 16, kw:kw + 32]
                            mm = nc.tensor.matmul(ps[:], lhsT, rhs,
                                                  start=(i == 0),
                                                  stop=(i == 26))
                            # keep the PE's static order group-contiguous so
                            # each store fires right after its last matmul
                            if i == 0 and prev_last_mm is not None:
                                tile.add_dep_helper(
                                    mm.ins, prev_last_mm.ins, sync=False,
                                    reason="group-contiguous PE order")
                            i += 1
                    prev_last_mm = mm
                    # PSUM cannot be DMA'd directly: fused bias-add + copy
                    # to SBUF on DVE, then store from GpSimd's queue
                    ot_g = ot_pool.tile([128, 512], dt.float32, tag=f"ot{g}")
                    nc.vector.tensor_scalar_add(ot_g[:], ps[:], bias_t[:])
                    nc.gpsimd.dma_start(out[:, g, :], ot_g[:])

    nc.compile()
    return nc


def _prep_shared(weight, bias):
    # wc[co, ci, kd, kh, kw] = weight[ci, co, 2-kd, 2-kh, 2-kw]
    wc = np.flip(np.transpose(weight, (1, 0, 2, 3, 4)), axis=(2, 3, 4))
    # full pre-built Toeplitz: tcw[dpi*64+ci, (c*9+t)*128 + co*4 + gb]
    tcw = np.zeros((128, 27, 128), np.float32)
    for c in range(3):
        for dpi in range(2):
            for gb in range(4):
                kd = 2 * c + dpi - gb
                if 0 <= kd <= 2:
                    arr = wc[:, :, kd].reshape(32, 64, 9).transpose(1, 2, 0)
                    tcw[dpi * 64:(dpi + 1) * 64,
                        c * 9:(c + 1) * 9, gb::4] = arr
    tcw = np.ascontiguousarray(tcw.reshape(128, 27 * 128))
    bias128 = np.ascontiguousarray(
        np.repeat(bias.astype(np.float32), 4).reshape(128, 1))
    return tcw, bias128


def _make_slab(x, n, cblk):
    # 5 pairs of spatially padded planes (34x34, zero border);
    # pair p = unpadded planes (8c-1+2p, 8c+2p)
    xs = np.zeros((5, 128, 34, 34), np.float32)
    lo = G_PER_CORE * cblk - 1
    for p in range(5):
        for dpi in range(2):
            d = lo + 2 * p + dpi
            if 0 <= d < 32:
                xs[p, dpi * 64:(dpi + 1) * 64, 1:33, 1:33] = x[n, :, d]
    return xs


def _make_in_maps(x, weight, bias):
    tcw, bias128 = _prep_shared(weight, bias)
    in_maps = []
    for core in range(N_CORES):
        n, cblk = divmod(core, D_BLOCKS)
        in_maps.append({"xs": _make_slab(x, n, cblk), "tcw": tcw,
                        "bias": bias128})
    return in_maps


def kernel(x, weight, bias):
    from concourse.bass_utils import run_bass_kernel_spmd

    if "nc" not in _cache:
        _cache["nc"] = _build_nc()
    nc = _cache["nc"]

    x = np.asarray(x, np.float32)
    weight = np.asarray(weight, np.float32)
    bias = np.asarray(bias, np.float32)

    in_maps = _make_in_maps(x, weight, bias)
    res = run_bass_kernel_spmd(nc, in_maps, core_ids=list(range(N_CORES)))

    # every non-conv voxel (even lattice positions, trailing output_padding
    # planes) is exactly bias; fill host-side and scatter the conv voxels
    # into the odd sub-lattice
    full = np.empty((2, 32, 66, 66, 66), np.float32)
    full[:] = bias.reshape(1, 32, 1, 1, 1)
    for core in range(N_CORES):
        n, cblk = divmod(core, D_BLOCKS)
        # [128, 4, 512] -> (co, gb, b, hh, h, w) -> (co, q=4b+gb, 16hh+h, w)
        arr = res.results[core]["out"].reshape(32, 4, 2, 2, 16, 32)
        arr = arr.transpose(0, 2, 1, 3, 4, 5).reshape(32, 8, 32, 32)
        d0 = 16 * cblk
        full[n, :, d0 + 1:d0 + 17:2, 1:65:2, 1:65:2] = arr
    return full
